# revision 1
# baseline (speedup 1.0000x reference)
import sys

sys.path.insert(0, "/opt/trn_rl_repo")

import numpy as np

import concourse.bass as bass
import concourse.bacc as bacc
import concourse.mybir as mybir
from concourse.tile import TileContext
from concourse.bass_utils import run_bass_kernel_spmd

P = 128          # partitions
BT = 512         # batch-tile (free dim) per matmul
G = 4            # batch groups packed into 128 partitions for the GRU
NCORES = 8
B, S, H, A = 131072, 256, 512, 32
BC = B // NCORES           # 16384 rows per core
MACRO = G * BT             # 2048 rows per GRU macro-tile
NM = BC // MACRO           # 8 macro-tiles per core

FP32 = mybir.dt.float32
AF = mybir.ActivationFunctionType
OP = mybir.AluOpType

_CACHE = {}


def _build(nsteps: int) -> bass.Bass:
    nc = bacc.Bacc("TRN2", target_bir_lowering=False, debug=False,
                   num_devices=NCORES)

    xT = nc.dram_tensor("xT", [S, BC], FP32, kind="ExternalInput")
    w1t = nc.dram_tensor("w1t", [S, H], FP32, kind="ExternalInput")
    w2t = nc.dram_tensor("w2t", [H, H], FP32, kind="ExternalInput")
    wmt = nc.dram_tensor("wmt", [H, A], FP32, kind="ExternalInput")
    b1d = nc.dram_tensor("b1d", [P, 4], FP32, kind="ExternalInput")
    b2d = nc.dram_tensor("b2d", [P, 4], FP32, kind="ExternalInput")
    bmd = nc.dram_tensor("bmd", [A, 1], FP32, kind="ExternalInput")
    lri = nc.dram_tensor("lri", [2 * G, P], FP32, kind="ExternalInput")
    lui = nc.dram_tensor("lui", [2 * G, P], FP32, kind="ExternalInput")
    lni = nc.dram_tensor("lni", [2 * G, P], FP32, kind="ExternalInput")
    lrh = nc.dram_tensor("lrh", [P, P], FP32, kind="ExternalInput")
    luh = nc.dram_tensor("luh", [P, P], FP32, kind="ExternalInput")
    lnh = nc.dram_tensor("lnh", [P, P], FP32, kind="ExternalInput")
    lwd = nc.dram_tensor("lwd", [P, 2 * G], FP32, kind="ExternalInput")
    brd = nc.dram_tensor("brd", [P, 1], FP32, kind="ExternalInput")
    bud = nc.dram_tensor("bud", [P, 1], FP32, kind="ExternalInput")
    bnhd = nc.dram_tensor("bnhd", [P, 1], FP32, kind="ExternalInput")
    bnid = nc.dram_tensor("bnid", [P, 1], FP32, kind="ExternalInput")
    bwd = nc.dram_tensor("bwd", [2 * G, 1], FP32, kind="ExternalInput")
    # packed device layout: [macro, 8t+2g+c, j]; host unscrambles to [B, 2T]
    outT = nc.dram_tensor("outT", [NM, 2 * G * nsteps, BT], FP32,
                          kind="ExternalOutput")

    xv = xT.rearrange("(kb p) b -> p kb b", p=P)              # [128, 2, BC]
    ov = outT

    with TileContext(nc) as tc:
        with (
            tc.tile_pool(name="const", bufs=1) as const,
            tc.tile_pool(name="xp", bufs=3) as xp,
            tc.tile_pool(name="actp", bufs=2) as actp,
            tc.tile_pool(name="grup", bufs=2) as grup,
            tc.tile_pool(name="outp", bufs=2) as outp,
            tc.tile_pool(name="mps", bufs=2, space="PSUM") as mps,
            tc.tile_pool(name="zps", bufs=1, space="PSUM") as zps,
            tc.tile_pool(name="gps", bufs=1, space="PSUM") as gps,
        ):
            w1s = const.tile([P, 2, H], FP32)
            nc.sync.dma_start(w1s[:], w1t.rearrange("(kb p) f -> p kb f", p=P))
            w2s = const.tile([P, 4, H], FP32)
            nc.sync.dma_start(w2s[:], w2t.rearrange("(kb p) f -> p kb f", p=P))
            wms = const.tile([P, 4, A], FP32)
            nc.sync.dma_start(wms[:], wmt.rearrange("(kb p) f -> p kb f", p=P))
            b1s = const.tile([P, 4], FP32)
            nc.sync.dma_start(b1s[:], b1d[:])
            b2s = const.tile([P, 4], FP32)
            nc.sync.dma_start(b2s[:], b2d[:])
            bms = const.tile([A, 1], FP32)
            nc.sync.dma_start(bms[:], bmd[:])
            lris = const.tile([2 * G, P], FP32)
            nc.sync.dma_start(lris[:], lri[:])
            luis = const.tile([2 * G, P], FP32)
            nc.sync.dma_start(luis[:], lui[:])
            lnis = const.tile([2 * G, P], FP32)
            nc.sync.dma_start(lnis[:], lni[:])
            lrhs = const.tile([P, P], FP32)
            nc.sync.dma_start(lrhs[:], lrh[:])
            luhs = const.tile([P, P], FP32)
            nc.sync.dma_start(luhs[:], luh[:])
            lnhs = const.tile([P, P], FP32)
            nc.sync.dma_start(lnhs[:], lnh[:])
            lws = const.tile([P, 2 * G], FP32)
            nc.sync.dma_start(lws[:], lwd[:])
            brs = const.tile([P, 1], FP32)
            nc.sync.dma_start(brs[:], brd[:])
            bus = const.tile([P, 1], FP32)
            nc.sync.dma_start(bus[:], bud[:])
            bnhs = const.tile([P, 1], FP32)
            nc.sync.dma_start(bnhs[:], bnhd[:])
            bnis = const.tile([P, 1], FP32)
            nc.sync.dma_start(bnis[:], bnid[:])
            bws = const.tile([2 * G, 1], FP32)
            nc.sync.dma_start(bws[:], bwd[:])

            for m in range(NM):
                Z = grup.tile([P, BT], FP32, tag="Z")
                for g in range(G):
                    c0 = m * MACRO + g * BT
                    X = xp.tile([P, 2, BT], FP32, tag="X")
                    nc.sync.dma_start(X[:], xv[:, :, c0:c0 + BT])
                    H1 = actp.tile([P, 4, BT], FP32, tag="H1")
                    for f in range(4):
                        ps = mps.tile([P, BT], FP32, tag="ps")
                        nc.tensor.matmul(ps[:], w1s[:, 0, f * P:(f + 1) * P],
                                         X[:, 0, :], start=True, stop=False)
                        nc.tensor.matmul(ps[:], w1s[:, 1, f * P:(f + 1) * P],
                                         X[:, 1, :], start=False, stop=True)
                        nc.scalar.activation(H1[:, f, :], ps[:], AF.Relu,
                                             bias=b1s[:, f:f + 1])
                    H2 = actp.tile([P, 4, BT], FP32, tag="H2")
                    for f in range(4):
                        ps = mps.tile([P, BT], FP32, tag="ps")
                        for k in range(4):
                            nc.tensor.matmul(ps[:], w2s[:, k, f * P:(f + 1) * P],
                                             H1[:, k, :], start=(k == 0),
                                             stop=(k == 3))
                        nc.scalar.activation(H2[:, f, :], ps[:], AF.Relu,
                                             bias=b2s[:, f:f + 1])
                    ps3 = zps.tile([A, BT], FP32, tag="ps3")
                    for k in range(4):
                        nc.tensor.matmul(ps3[:], wms[:, k, :], H2[:, k, :],
                                         start=(k == 0), stop=(k == 3))
                    nc.scalar.activation(Z[g * A:(g + 1) * A, :], ps3[:],
                                         AF.Identity, bias=bms[:, :1])

                WP = grup.tile([2 * G, BT], FP32, tag="WP")
                nc.any.memset(WP[:], 0.0)
                wp_cur = WP[:]
                for t in range(nsteps):
                    psR = gps.tile([P, BT], FP32, tag="psR")
                    psU = gps.tile([P, BT], FP32, tag="psU")
                    psNI = gps.tile([P, BT], FP32, tag="psNI")
                    psNH = gps.tile([P, BT], FP32, tag="psNH")
                    nc.tensor.matmul(psR[:], lris[:], wp_cur, start=True, stop=False)
                    nc.tensor.matmul(psR[:], lrhs[:], Z[:], start=False, stop=True)
                    nc.tensor.matmul(psU[:], luis[:], wp_cur, start=True, stop=False)
                    nc.tensor.matmul(psU[:], luhs[:], Z[:], start=False, stop=True)
                    nc.tensor.matmul(psNI[:], lnis[:], wp_cur, start=True, stop=True)
                    nc.tensor.matmul(psNH[:], lnhs[:], Z[:], start=True, stop=True)
                    R = grup.tile([P, BT], FP32, tag="R")
                    U = grup.tile([P, BT], FP32, tag="U")
                    HN = grup.tile([P, BT], FP32, tag="HN")
                    NT = grup.tile([P, BT], FP32, tag="NT")
                    nc.scalar.activation(R[:], psR[:], AF.Sigmoid, bias=brs[:, :1])
                    nc.scalar.activation(U[:], psU[:], AF.Sigmoid, bias=bus[:, :1])
                    nc.scalar.activation(HN[:], psNH[:], AF.Identity, bias=bnhs[:, :1])
                    nc.vector.tensor_tensor(R[:], R[:], HN[:], OP.mult)
                    nc.vector.tensor_tensor(R[:], R[:], psNI[:], OP.add)
                    nc.scalar.activation(NT[:], R[:], AF.Tanh, bias=bnis[:, :1])
                    nc.vector.tensor_tensor(Z[:], Z[:], NT[:], OP.subtract)
                    nc.vector.tensor_tensor(Z[:], U[:], Z[:], OP.mult)
                    nc.vector.tensor_tensor(Z[:], Z[:], NT[:], OP.add)
                    psW = gps.tile([2 * G, BT], FP32, tag="psW")
                    nc.tensor.matmul(psW[:], lws[:], Z[:], start=True, stop=True)
                    wp_next = outp.tile([2 * G, BT], FP32, tag="WPN")
                    nc.scalar.activation(wp_next[:], psW[:], AF.Identity,
                                         bias=bws[:, :1])
                    nc.vector.tensor_tensor(wp_next[:], wp_next[:], wp_cur, OP.add)
                    nc.sync.dma_start(ov[m, 2 * G * t:2 * G * (t + 1), :],
                                      wp_next[:])
                    wp_cur = wp_next[:]
    nc.compile()
    return nc


LAST_RESULT = None


def kernel(**inputs) -> np.ndarray:
    global LAST_RESULT
    x = np.ascontiguousarray(np.asarray(inputs["x"], dtype=np.float32))
    W1 = np.asarray(inputs["W1"], np.float32)
    b1 = np.asarray(inputs["b1"], np.float32)
    W2 = np.asarray(inputs["W2"], np.float32)
    b2 = np.asarray(inputs["b2"], np.float32)
    Wm = np.asarray(inputs["Wm"], np.float32)
    bm = np.asarray(inputs["bm"], np.float32)
    w_ih = np.asarray(inputs["w_ih"], np.float32)
    w_hh = np.asarray(inputs["w_hh"], np.float32)
    b_ih = np.asarray(inputs["b_ih"], np.float32)
    b_hh = np.asarray(inputs["b_hh"], np.float32)
    Ww = np.asarray(inputs["Ww"], np.float32)
    bw = np.asarray(inputs["bw"], np.float32)
    T = int(inputs["pred_length"])

    I4 = np.eye(G, dtype=np.float32)
    common = {
        "w1t": np.ascontiguousarray(W1.T),
        "w2t": np.ascontiguousarray(W2.T),
        "wmt": np.ascontiguousarray(Wm.T),
        "b1d": np.ascontiguousarray(b1.reshape(4, P).T),
        "b2d": np.ascontiguousarray(b2.reshape(4, P).T),
        "bmd": bm.reshape(A, 1).copy(),
        "lri": np.ascontiguousarray(np.kron(I4, w_ih[0:A].T)),
        "lui": np.ascontiguousarray(np.kron(I4, w_ih[A:2 * A].T)),
        "lni": np.ascontiguousarray(np.kron(I4, w_ih[2 * A:3 * A].T)),
        "lrh": np.ascontiguousarray(np.kron(I4, w_hh[0:A].T)),
        "luh": np.ascontiguousarray(np.kron(I4, w_hh[A:2 * A].T)),
        "lnh": np.ascontiguousarray(np.kron(I4, w_hh[2 * A:3 * A].T)),
        "lwd": np.ascontiguousarray(np.kron(I4, Ww.T)),
        "brd": np.tile(b_ih[0:A] + b_hh[0:A], G).reshape(P, 1).copy(),
        "bud": np.tile(b_ih[A:2 * A] + b_hh[A:2 * A], G).reshape(P, 1).copy(),
        "bnhd": np.tile(b_hh[2 * A:3 * A], G).reshape(P, 1).copy(),
        "bnid": np.tile(b_ih[2 * A:3 * A], G).reshape(P, 1).copy(),
        "bwd": np.tile(bw, G).reshape(2 * G, 1).copy(),
    }
    global _last_common
    _last_common = common
    xT = np.ascontiguousarray(x.T)          # [S, B]
    in_maps = []
    for i in range(NCORES):
        m = dict(common)
        m["xT"] = np.ascontiguousarray(xT[:, i * BC:(i + 1) * BC])
        in_maps.append(m)

    if T not in _CACHE:
        _CACHE[T] = _build(T)
    nc = _CACHE[T]
    res = run_bass_kernel_spmd(nc, in_maps, core_ids=list(range(NCORES)))
    LAST_RESULT = res
    parts = []
    for i in range(NCORES):
        o = np.asarray(res.results[i]["outT"])       # [NM, 2*G*T, BT]
        o = o.reshape(NM, T, G, 2, BT).transpose(0, 2, 4, 1, 3)
        parts.append(o.reshape(BC, 2 * T))
    return np.ascontiguousarray(np.concatenate(parts, axis=0))

